# revision 1
# baseline (speedup 1.0000x reference)
"""Trainium2 Bass kernel for nn_MinimalNetwork (equivariant GNN message passing).

Fully fused per-edge pipeline, sharded over 8 NeuronCores by edge (data
parallel). Each core:
  radial-basis -> 3-layer silu MLP -> R [e,1216] (TensorE, PSUM-resident)
  CY = rsh @ CC2 (CG coefficients folded into one constant matmul)
  D-stage: 72 per-edge-scalar MACs (VectorE scalar_tensor_tensor)
  R-stage: 9 broadcast products + halving-tree (v,k) reduction (VectorE)
  scatter: DMA-CCE scatter-add of per-edge messages into the node table.
Host sums the 8 per-core node tables.

Self-contained: all shapes/layouts hardcoded for the 200000-edge / 12500-node
problem instance (works for any multiple-of-512 edge shard; see kernel()).
"""

import math
from contextlib import ExitStack
from itertools import accumulate

import numpy as np

# ----------------- problem constants (hardcoded) -----------------
N_NODES = 12500
N_EDGES = 200000
N_CORES = 8
RS = [(8, 0), (8, 1), (8, 2)]
LO = [0, 1, 2]
SH_DIM = 25
FEAT_OFF = [0] + list(accumulate(m * (2 * l + 1) for m, l in RS))
FEAT_DIM = FEAT_OFF[-1]  # 72
R_OFF = [0] + list(
    accumulate(mo * mi * (2 * min(lo, li) + 1) for mo, lo in RS for mi, li in RS)
)
R_DIM = R_OFF[-1]  # 1216
N_BASIS, H = 10, 100
MIN_R, MAX_R = 0.7, 3.2
SWISH_SCALE = 1.679177
SUB = 128          # edges per sub-tile (partition dim)
SUPER = 512        # edges per super-tile (MLP batch)
N_SUB = SUPER // SUB


def _pair_nl(i, j):
    return 2 * min(LO[i], LO[j]) + 1


def _wj(j):
    return sum(_pair_nl(i, j) * (2 * LO[i] + 1) for i in range(3))


W_J = [_wj(j) for j in range(3)]  # [9, 25, 35]


def _off_ij(i, j):
    return sum(_pair_nl(i2, j) * (2 * LO[i2] + 1) for i2 in range(i))


# uniform padded D layout: every j-block is [v(8) x WU(35)], stride 280
WU = max(W_J)
D_JOFF = [8 * WU * j for j in range(4)]
D_DIM = D_JOFF[-1]  # 840
CY_JOFF = [0] + list(accumulate((2 * LO[j] + 1) * W_J[j] for j in range(3)))
CY_DIM = CY_JOFF[-1]  # 259


def _cc_layout():
    layout, off = {}, 0
    for _, lo in RS:
        for _, li in RS:
            for lf in range(abs(lo - li), lo + li + 1):
                if (lo, li, lf) not in layout:
                    shp = (2 * lo + 1, 2 * li + 1, 2 * lf + 1)
                    layout[(lo, li, lf)] = (off, shp)
                    off += shp[0] * shp[1] * shp[2]
    return layout, off


CC_LAYOUT, CC_TOTAL = _cc_layout()  # 1225


def _norm_coef():
    nc = np.zeros((3, 3), dtype=np.float64)
    for i, (_, lo) in enumerate(RS):
        ns = sum(mi * (2 * min(lo, li) + 1) for mi, li in RS)
        nc[i, :] = math.sqrt(4 * math.pi) * math.sqrt(2 * lo + 1) / math.sqrt(ns)
    return nc


NORM = _norm_coef()


def build_cc2(cc: np.ndarray) -> np.ndarray:
    """CC2 [25, CY_DIM]: CY[e,:] = rsh[e,:] @ CC2 (NORM folded in)."""
    cc2 = np.zeros((SH_DIM, CY_DIM), dtype=np.float32)
    for j in range(3):
        lj = LO[j]
        for ii in range(2 * lj + 1):
            blk = CY_JOFF[j] + ii * W_J[j]
            for i in range(3):
                lo = LO[i]
                no = 2 * lo + 1
                base = blk + _off_ij(i, j)
                for k, lf in enumerate(range(abs(lo - lj), lo + lj + 1)):
                    off, shp = CC_LAYOUT[(lo, lj, lf)]
                    C = cc[off : off + shp[0] * shp[1] * shp[2]].reshape(shp)
                    for o in range(no):
                        col = base + k * no + o
                        cc2[lf * lf : lf * lf + 2 * lf + 1, col] = (
                            np.float32(NORM[i, j]) * C[o, ii, :]
                        )
    return cc2


def fold_weights(W0, W1, W2, W3):
    s = SWISH_SCALE
    return (
        (W0 / math.sqrt(N_BASIS)).astype(np.float32),
        (s * W1 / math.sqrt(H)).astype(np.float32),
        (s * W2 / math.sqrt(H)).astype(np.float32),
        (s * W3 / math.sqrt(H)).astype(np.float32),
    )


# ----------------- bass program -----------------

def build_program(e_pad: int, n_nodes: int):
    import concourse.bass as bass
    import concourse.tile as tile
    from concourse import bacc, mybir

    f32 = mybir.dt.float32
    i32 = mybir.dt.int32
    AF = mybir.ActivationFunctionType
    OP = mybir.AluOpType

    n_super = e_pad // SUPER
    assert e_pad % SUPER == 0

    nc = bacc.Bacc()

    # DRAM tensors (per-core inputs)
    rshT_d = nc.dram_tensor("rshT", [SH_DIM, e_pad], f32, kind="ExternalInput")
    radii_d = nc.dram_tensor("radii", [1, e_pad], f32, kind="ExternalInput")
    src_d = nc.dram_tensor("srcidx", [e_pad, 1], i32, kind="ExternalInput")
    dst_d = nc.dram_tensor("dstidx", [e_pad, 1], i32, kind="ExternalInput")
    dstf_d = nc.dram_tensor("dstf", [e_pad, 1], f32, kind="ExternalInput")
    feat_d = nc.dram_tensor("features", [n_nodes, FEAT_DIM], f32, kind="ExternalInput")
    w0_d = nc.dram_tensor("W0p", [N_BASIS, H], f32, kind="ExternalInput")
    w1_d = nc.dram_tensor("W1p", [H, H], f32, kind="ExternalInput")
    w2_d = nc.dram_tensor("W2p", [H, H], f32, kind="ExternalInput")
    w3_d = nc.dram_tensor("W3p", [H, R_DIM], f32, kind="ExternalInput")
    cc2_d = nc.dram_tensor("CC2", [SH_DIM, CY_DIM], f32, kind="ExternalInput")
    csc_d = nc.dram_tensor("cscale", [N_BASIS, 1], f32, kind="ExternalInput")
    cbi_d = nc.dram_tensor("cbias", [N_BASIS, 1], f32, kind="ExternalInput")
    # +1 dummy row: pad edges scatter zeros there
    out_d = nc.dram_tensor("out", [n_nodes + 1, FEAT_DIM], f32, kind="ExternalOutput")

    with tile.TileContext(nc) as tc, ExitStack() as ctx:
        cpool = ctx.enter_context(tc.tile_pool(name="consts", bufs=1))
        inpool = ctx.enter_context(tc.tile_pool(name="in", bufs=3))
        hpool = ctx.enter_context(tc.tile_pool(name="h", bufs=2))
        dpool = ctx.enter_context(tc.tile_pool(name="work", bufs=2))
        rtpool = ctx.enter_context(tc.tile_pool(name="rtmp", bufs=2))
        mpool = ctx.enter_context(tc.tile_pool(name="msg", bufs=3))
        ps_mlp = ctx.enter_context(tc.tile_pool(name="psmlp", bufs=1, space="PSUM"))
        ps_r = ctx.enter_context(tc.tile_pool(name="psr", bufs=1, space="PSUM"))
        ps_cy = ctx.enter_context(tc.tile_pool(name="pscy", bufs=1, space="PSUM"))
        ps_bc = ctx.enter_context(tc.tile_pool(name="psbc", bufs=1, space="PSUM"))
        ps_cmb = ctx.enter_context(tc.tile_pool(name="pscmb", bufs=1, space="PSUM"))

        # constants into SBUF
        w0_s = cpool.tile([N_BASIS, H], f32)
        w1_s = cpool.tile([H, H], f32)
        w2_s = cpool.tile([H, H], f32)
        w3_s = cpool.tile([H, R_DIM], f32)
        cc2_s = cpool.tile([SH_DIM, CY_DIM], f32)
        csc_s = cpool.tile([N_BASIS, 1], f32)
        cbi_s = cpool.tile([N_BASIS, 1], f32)
        ones_s = cpool.tile([1, N_BASIS], f32)
        zero_s = cpool.tile([SUB, FEAT_DIM], f32)
        ident_s = cpool.tile([SUB, SUB], f32)
        for t, d in (
            (w0_s, w0_d), (w1_s, w1_d), (w2_s, w2_d), (w3_s, w3_d),
            (cc2_s, cc2_d), (csc_s, csc_d), (cbi_s, cbi_d),
        ):
            nc.sync.dma_start(t[:], d[:])
        nc.vector.memset(ones_s[:], 1.0)
        nc.vector.memset(zero_s[:], 0.0)
        from concourse.masks import make_identity
        make_identity(nc, ident_s[:])

        # zero-init the output table (n_nodes + 1 rows)
        n_out = n_nodes + 1
        nfull = n_out // SUB
        if nfull:
            nc.sync.dma_start(
                out_d[: nfull * SUB, :].rearrange("(a p) c -> p a c", p=SUB),
                zero_s[:].unsqueeze(1).broadcast_to((SUB, nfull, FEAT_DIM)),
            )
        rem = n_out - nfull * SUB
        if rem:
            nc.sync.dma_start(out_d[nfull * SUB :, :], zero_s[:rem, :])

        for s in range(n_super):
            e0 = s * SUPER
            # ---- loads ----
            rsh_t = inpool.tile([SH_DIM, SUPER], f32, tag="rsh")
            nc.sync.dma_start(rsh_t[:], rshT_d[:, e0 : e0 + SUPER])
            rad_t = inpool.tile([1, SUPER], f32, tag="rad")
            nc.sync.dma_start(rad_t[:], radii_d[:, e0 : e0 + SUPER])
            src_t = inpool.tile([SUB, N_SUB], i32, tag="src")
            nc.sync.dma_start(
                src_t[:],
                src_d[e0 : e0 + SUPER, 0].rearrange("(c p) -> p c", p=SUB),
            )
            dst_t = inpool.tile([SUB, N_SUB], i32, tag="dst")
            nc.sync.dma_start(
                dst_t[:],
                dst_d[e0 : e0 + SUPER, 0].rearrange("(c p) -> p c", p=SUB),
            )
            dstf_t = inpool.tile([SUB, N_SUB], f32, tag="dstf")
            nc.sync.dma_start(
                dstf_t[:],
                dstf_d[e0 : e0 + SUPER, 0].rearrange("(c p) -> p c", p=SUB),
            )
            fg_t = inpool.tile([SUB, N_SUB * FEAT_DIM], f32, tag="fg")
            for c in range(N_SUB):
                nc.gpsimd.indirect_dma_start(
                    out=fg_t[:, c * FEAT_DIM : (c + 1) * FEAT_DIM],
                    out_offset=None,
                    in_=feat_d[:],
                    in_offset=bass.IndirectOffsetOnAxis(ap=src_t[:, c : c + 1], axis=0),
                )

            # ---- radial basis ----
            rb_ps = ps_bc.tile([N_BASIS, SUPER], f32, tag="bc", space="PSUM")
            nc.tensor.matmul(rb_ps[:], ones_s[:], rad_t[:], start=True, stop=True)
            z2_t = hpool.tile([N_BASIS, SUPER], f32, tag="z2")
            nc.scalar.activation(
                z2_t[:], rb_ps[:], AF.Square, bias=cbi_s[:], scale=csc_s[:]
            )
            bas_t = hpool.tile([N_BASIS, SUPER], f32, tag="bas")
            nc.scalar.activation(bas_t[:], z2_t[:], AF.Exp, scale=-1.0)

            # ---- MLP ----
            h = bas_t
            for li, w_s in enumerate((w0_s, w1_s, w2_s)):
                hp = ps_mlp.tile([H, SUPER], f32, tag="hp", space="PSUM")
                nc.tensor.matmul(hp[:], w_s[:], h[:], start=True, stop=True)
                hn = hpool.tile([H, SUPER], f32, tag=f"h{li}")
                nc.scalar.activation(hn[:], hp[:], AF.Silu)
                h = hn

            for c in range(N_SUB):
                esl = slice(c * SUB, (c + 1) * SUB)
                # ---- R = h3_c^T @ W3p  -> PSUM [128, 1216] ----
                r_ps = ps_r.tile([SUB, R_DIM], f32, tag="r", space="PSUM")
                for n0 in range(0, R_DIM, 512):
                    n1 = min(n0 + 512, R_DIM)
                    nc.tensor.matmul(
                        r_ps[:, n0:n1], h[:, esl], w3_s[:, n0:n1],
                        start=True, stop=True,
                    )
                r_sb = dpool.tile([SUB, R_DIM], f32, tag="rsb")
                nc.scalar.copy(r_sb[:], r_ps[:])
                # ---- CY ----
                cy_ps = ps_cy.tile([SUB, CY_DIM], f32, tag="cy", space="PSUM")
                nc.tensor.matmul(
                    cy_ps[:], rsh_t[:, esl], cc2_s[:], start=True, stop=True
                )
                cy_t = dpool.tile([SUB, CY_DIM], f32, tag="cys")
                nc.scalar.copy(cy_t[:], cy_ps[:])

                # ---- D-stage: per j: one broadcast product + one reduce ----
                # Dtmp_j[v, w, ii] = F[(v,ii)] * CY[(ii, w)]; D_j[v, w] = sum_ii
                d_t = dpool.tile([SUB, D_DIM], f32, tag="d")
                for j in range(3):
                    nj = 2 * LO[j] + 1
                    w = W_J[j]
                    f_ap = (
                        fg_t[:, c * FEAT_DIM + FEAT_OFF[j] :
                             c * FEAT_DIM + FEAT_OFF[j + 1]]
                        .rearrange("p (v i) -> p v i", v=8)
                        .unsqueeze(3)
                        .broadcast_to((SUB, 8, nj, w))
                    )
                    cy_ap = (
                        cy_t[:, CY_JOFF[j] : CY_JOFF[j + 1]]
                        .rearrange("p (i w) -> p i w", i=nj)
                        .unsqueeze(1)
                        .broadcast_to((SUB, 8, nj, w))
                    )
                    dj = d_t[:, D_JOFF[j] : D_JOFF[j + 1]].rearrange(
                        "p (v w) -> p v w", w=WU
                    )[:, :, :w]
                    if nj == 1:
                        nc.vector.tensor_tensor(dj.unsqueeze(2), f_ap, cy_ap, OP.mult)
                    else:
                        dtmp = dpool.tile([SUB, 8 * w * nj], f32, tag=f"dt{j}")
                        # layout [v, w, ii] (ii innermost for the reduce);
                        # product iterates (v, ii, w)
                        out_ap = (
                            dtmp[:]
                            .rearrange("p (v w i) -> p v w i", v=8, w=w)
                            .transpose((0, 1, 3, 2))
                        )
                        nc.vector.tensor_tensor(out_ap, f_ap, cy_ap, OP.mult)
                        nc.vector.tensor_reduce(
                            dj,
                            dtmp[:].rearrange("p (v w i) -> p v w i", v=8, i=nj),
                            mybir.AxisListType.X,
                            OP.add,
                        )

                # ---- R-stage: per i: 3 products into joint [u, o, m] buffer,
                # one innermost-m reduce -> msg block ----
                msg_t = mpool.tile([SUB, FEAT_DIM], f32, tag="msg")
                for i in range(3):
                    lo = LO[i]
                    no = 2 * lo + 1
                    m_i = 8 * sum(_pair_nl(i, j) for j in range(3))
                    rt = rtpool.tile([SUB, 8 * no * m_i], f32, tag=f"rt{i}")
                    rt4 = rt[:].rearrange(
                        "p (u o m) -> p u o m", u=8, o=no
                    )
                    if i == 0:
                        # all three pairs have nl=1, no=1 and identical shapes:
                        # one joint product over (j, u, v)
                        r_ap = r_sb[:, R_OFF[0] : R_OFF[3]].rearrange(
                            "p (j u v) -> p j u v", j=3, u=8
                        )
                        d_ap = (
                            d_t[:]
                            .rearrange("p (j v w) -> p j v w", j=3, v=8)[:, :, :, 0]
                            .unsqueeze(2)
                            .broadcast_to((SUB, 3, 8, 8))
                        )
                        out_ap = rt[:].rearrange(
                            "p (u j v) -> p j u v", u=8, j=3
                        )
                        nc.vector.tensor_tensor(out_ap, r_ap, d_ap, OP.mult)
                        groups = []
                    elif i == 1:
                        groups = [(3, [0]), (4, [1]), (5, [2])]
                    else:
                        groups = [(6, [0]), (7, [1]), (8, [2])]
                    for p0, js in groups:
                        j = js[0]
                        nl = _pair_nl(i, j)
                        moff = 8 * sum(_pair_nl(i, j2) for j2 in range(j))
                        r_ap = (
                            r_sb[:, R_OFF[p0] : R_OFF[p0 + 1]]
                            .rearrange("p (u v k) -> p u v k", u=8, v=8)
                            .unsqueeze(4)
                            .broadcast_to((SUB, 8, 8, nl, no))
                        )
                        oij = _off_ij(i, j)
                        d_ap = (
                            d_t[:, D_JOFF[j] : D_JOFF[j + 1]]
                            .rearrange("p (v w) -> p v w", v=8)[
                                :, :, oij : oij + nl * no
                            ]
                            .rearrange("p v (k o) -> p v k o", k=nl)
                            .unsqueeze(1)
                            .broadcast_to((SUB, 8, 8, nl, no))
                        )
                        out_ap = (
                            rt4[:, :, :, moff : moff + 8 * nl]
                            .rearrange("p u o (v k) -> p u o v k", v=8)
                            .transpose((0, 1, 3, 4, 2))
                        )
                        nc.vector.tensor_tensor(out_ap, r_ap, d_ap, OP.mult)
                    mb = msg_t[:, FEAT_OFF[i] : FEAT_OFF[i + 1]]
                    nc.vector.tensor_reduce(
                        mb,
                        rt[:].rearrange("p (g m) -> p g m", m=m_i),
                        mybir.AxisListType.X,
                        OP.add,
                    )

                # ---- combine duplicate-dst rows, then scatter (plain write) ----
                # sel[p,q] = (dst[p] == dst[q]); msg2 = sel @ msg sums each
                # dst-group into every one of its rows, so colliding DMA
                # writes all carry identical values. Host guarantees a dst
                # never straddles a 128-edge tile.
                tp_ps = ps_cmb.tile([SUB, SUB], f32, tag="tp", space="PSUM")
                nc.tensor.transpose(
                    tp_ps[:],
                    dstf_t[:, c : c + 1].to_broadcast((SUB, SUB)),
                    ident_s[:],
                )
                dstT_t = mpool.tile([SUB, SUB], f32, tag="dstT")
                nc.scalar.copy(dstT_t[:], tp_ps[:])
                sel_t = mpool.tile([SUB, SUB], f32, tag="sel")
                nc.vector.tensor_tensor(
                    sel_t[:],
                    dstf_t[:, c : c + 1].to_broadcast((SUB, SUB)),
                    dstT_t[:],
                    OP.is_equal,
                )
                cmb_ps = ps_cmb.tile([SUB, FEAT_DIM], f32, tag="cmb", space="PSUM")
                nc.tensor.matmul(
                    cmb_ps[:], sel_t[:], msg_t[:], start=True, stop=True
                )
                msg2_t = mpool.tile([SUB, FEAT_DIM], f32, tag="msg2")
                nc.scalar.copy(msg2_t[:], cmb_ps[:])
                nc.gpsimd.indirect_dma_start(
                    out=out_d[:],
                    out_offset=bass.IndirectOffsetOnAxis(
                        ap=dst_t[:, c : c + 1], axis=0
                    ),
                    in_=msg2_t[:],
                    in_offset=None,
                )

    nc.finalize()
    return nc


# ----------------- host side -----------------

def _prep_consts(cc, W0, W1, W2, W3):
    W0p, W1p, W2p, W3p = fold_weights(W0, W1, W2, W3)
    cc2 = build_cc2(np.asarray(cc, dtype=np.float32))
    centers = np.linspace(MIN_R, MAX_R, N_BASIS).astype(np.float32)
    spacing = (MAX_R - MIN_R) / (N_BASIS - 1)
    cscale = np.full((N_BASIS, 1), 1.0 / spacing, dtype=np.float32)
    cbias = (-centers / spacing).astype(np.float32).reshape(N_BASIS, 1)
    return W0p, W1p, W2p, W3p, cc2, cscale, cbias


def pack_edges(dst: np.ndarray, n_nodes: int):
    """Group edges by dst and bin-pack the per-dst groups into 128-edge
    tiles (best-fit decreasing) so no dst's edge-group straddles a tile.
    Returns int64 array [n_tiles, SUB] of original edge ids, -1 for pads."""
    import bisect

    order = np.argsort(dst, kind="stable")
    ds = dst[order]
    starts = np.flatnonzero(np.r_[True, ds[1:] != ds[:-1]])
    ends = np.r_[starts[1:], len(ds)]
    runs = sorted(
        ((int(e - s), int(s), int(e)) for s, e in zip(starts, ends)),
        key=lambda r: -r[0],
    )
    assert runs[0][0] <= SUB, f"node with {runs[0][0]} > {SUB} in-edges"
    bins = []   # each: list of (s, e) sorted-run slices
    rems = []   # ascending remaining capacities, parallel with binidx
    binidx = []
    for L, s, e in runs:
        k = bisect.bisect_left(rems, L)
        if k == len(rems):
            bins.append([(s, e)])
            r, bi = SUB - L, len(bins) - 1
        else:
            bi = binidx[k]
            r = rems[k] - L
            del rems[k], binidx[k]
            bins[bi].append((s, e))
        j = bisect.bisect_left(rems, r)
        rems.insert(j, r)
        binidx.insert(j, bi)
    tiles = []
    for b in bins:
        cur = []
        for s, e in b:
            cur.extend(order[s:e].tolist())
        cur.extend([-1] * (SUB - len(cur)))
        tiles.append(cur)
    return np.array(tiles, dtype=np.int64)


def _build_and_maps(edge_index, features, radii, rsh, cc, W0, W1, W2, W3):
    edge_index = np.asarray(edge_index)
    features = np.ascontiguousarray(np.asarray(features, dtype=np.float32))
    radii = np.asarray(radii, dtype=np.float32)
    rsh = np.ascontiguousarray(np.asarray(rsh, dtype=np.float32))
    n_nodes = features.shape[0]
    E = radii.shape[0]

    W0p, W1p, W2p, W3p, cc2, cscale, cbias = _prep_consts(cc, W0, W1, W2, W3)

    src = edge_index[0].astype(np.int64)
    dst = edge_index[1].astype(np.int64)
    tiles = pack_edges(dst, n_nodes)
    n_tiles = tiles.shape[0]

    n_cores = N_CORES
    tiles_per_core = -(-n_tiles // n_cores)
    # round up to a whole number of super-tiles
    tpc = -(-tiles_per_core // N_SUB) * N_SUB
    e_pad = tpc * SUB

    nc = build_program(e_pad, n_nodes)

    in_maps = []
    for k in range(n_cores):
        sel = tiles[k * tiles_per_core : (k + 1) * tiles_per_core]
        flat = sel.reshape(-1)
        flat = np.concatenate([flat, np.full(e_pad - flat.size, -1, np.int64)])
        valid = flat >= 0
        idx = np.where(valid, flat, 0)

        rshT_s = np.ascontiguousarray(
            np.where(valid[None, :], rsh.T[:, idx], np.float32(0.0))
        ).astype(np.float32)
        radii_s = np.where(valid, radii[idx], np.float32(1.0)).reshape(1, -1)
        radii_s = np.ascontiguousarray(radii_s).astype(np.float32)
        src_s = np.where(valid, src[idx], 0).astype(np.int32).reshape(-1, 1)
        dst_v = np.where(valid, dst[idx], n_nodes)
        dst_s = dst_v.astype(np.int32).reshape(-1, 1)
        dstf_s = dst_v.astype(np.float32).reshape(-1, 1)
        in_maps.append(
            dict(
                rshT=rshT_s,
                radii=radii_s,
                srcidx=np.ascontiguousarray(src_s),
                dstidx=np.ascontiguousarray(dst_s),
                dstf=np.ascontiguousarray(dstf_s),
                features=features,
                W0p=W0p, W1p=W1p, W2p=W2p, W3p=W3p,
                CC2=cc2, cscale=cscale, cbias=cbias,
            )
        )

    return nc, in_maps, n_nodes


def kernel(edge_index, features, radii, rsh, cc, W0, W1, W2, W3):
    from concourse.bass_utils import run_bass_kernel_spmd

    nc, in_maps, n_nodes = _build_and_maps(
        edge_index, features, radii, rsh, cc, W0, W1, W2, W3
    )
    res = run_bass_kernel_spmd(nc, in_maps, core_ids=list(range(N_CORES)))
    out = np.zeros((n_nodes, FEAT_DIM), dtype=np.float32)
    for r in res.results:
        out += r["out"][:n_nodes]
    return out


def _install_ntff_shim():
    """Provide antenv.axon_hooks + the ctypes NTFF hook if absent."""
    import contextlib
    import ctypes
    import sys
    import types

    try:
        from antenv.axon_hooks import get_axon_ntff_profile_hook  # noqa: F401
        return
    except ImportError:
        pass

    holder = {}
    mod = types.ModuleType("antenv.axon_hooks")
    mod.set_axon_ntff_profile_hook = lambda h: holder.__setitem__("h", h)
    mod.get_axon_ntff_profile_hook = lambda: holder.get("h")
    import antenv

    sys.modules["antenv.axon_hooks"] = mod
    antenv.axon_hooks = mod

    so_path = "/opt/axon/libaxon_pjrt.so"
    try:
        lib = ctypes.CDLL(so_path)
    except OSError:
        return
    if not hasattr(lib, "axon_start_nrt_profile"):
        return
    lib.axon_start_nrt_profile.argtypes = [
        ctypes.POINTER(ctypes.c_int64),
        ctypes.c_size_t,
    ]
    lib.axon_start_nrt_profile.restype = ctypes.c_int64
    lib.axon_stop_nrt_profile.argtypes = [ctypes.c_char_p]
    lib.axon_stop_nrt_profile.restype = ctypes.c_int64

    @contextlib.contextmanager
    def _hook(output_dir, device_ids):
        import jax

        jax.devices()
        if device_ids:
            ids = (ctypes.c_int64 * len(device_ids))(*device_ids)
            rc = lib.axon_start_nrt_profile(ids, len(device_ids))
        else:
            rc = lib.axon_start_nrt_profile(None, 0)
        if rc != 0:
            raise RuntimeError(f"axon_start_nrt_profile rc={rc}")
        try:
            yield
        finally:
            n = lib.axon_stop_nrt_profile(str(output_dir).encode())
            print(f"ntff profile: {n} file(s) written to {output_dir}")

    mod.set_axon_ntff_profile_hook(_hook)


def kernel_traced(edge_index, features, radii, rsh, cc, W0, W1, W2, W3,
                  trace_cores=None, tmpdir=None):
    """Run with NTFF tracing; returns BassKernelResults."""
    _install_ntff_shim()
    from concourse import bass_utils

    # no artifact bucket in this container
    bass_utils.upload_artifacts = lambda d: f"local:{d}"

    nc, in_maps, n_nodes = _build_and_maps(
        edge_index, features, radii, rsh, cc, W0, W1, W2, W3
    )
    return bass_utils.run_bass_kernel_spmd(
        nc, in_maps, core_ids=list(range(N_CORES)), trace=True,
        trace_cores=trace_cores, tmpdir=tmpdir,
    )



# revision 7
# speedup vs baseline: 1.3611x; 1.3611x over previous
"""Trainium2 Bass kernel for nn_MinimalNetwork (equivariant GNN message passing).

v2 — PE-centric restructure of the per-edge tensor product:
  * all matmuls bf16 (fp32 streams 4 cyc/col on TRN2 PE, bf16 1 cyc/col)
  * features pre-gathered on host into an ii-padded bf16 layout so the
    D-stage products run in the DVE 2x_1p bf16 mode (innermost step-1)
  * D-stage reduction over ii: bf16 halving-tree adds (tensor_reduce is 1x-only)
  * R-stage reduction over (v,k), duplicate-dst combining AND the per-tile
    segment-sum are all ONE PSUM-accumulated matmul chain  out = A^T @ rt
    with a host-built 0/1 edge->slot assignment matrix A per 128-edge tile
  * scatter writes only distinct-dst rows (slot table), pads hit a dummy row

Sharded over 8 NeuronCores by edge (data parallel); host sums the per-core
node tables.  Self-contained: shapes hardcoded for the 200000-edge /
12500-node instance (any multiple-of-512 edge shard works).
"""

import math
from contextlib import ExitStack
from itertools import accumulate

import numpy as np
import ml_dtypes

BF16 = ml_dtypes.bfloat16

# ----------------- problem constants (hardcoded) -----------------
N_NODES = 12500
N_EDGES = 200000
N_CORES = 8
RS = [(8, 0), (8, 1), (8, 2)]
LO = [0, 1, 2]
SH_DIM = 25
FEAT_OFF = [0] + list(accumulate(m * (2 * l + 1) for m, l in RS))
FEAT_DIM = FEAT_OFF[-1]  # 72
R_OFF_OLD = [0] + list(
    accumulate(mo * mi * (2 * min(lo, li) + 1) for mo, lo in RS for mi, li in RS)
)
N_BASIS, H = 10, 100
MIN_R, MAX_R = 0.7, 3.2
SWISH_SCALE = 1.679177
SUB = 128
SUPER = 512
N_SUB = SUPER // SUB

NO = [1, 3, 5]                    # 2*lo+1 per output i
NJ = [1, 3, 5]                    # 2*li+1 per input j
NL = [[2 * min(i, j) + 1 for j in range(3)] for i in range(3)]  # nl[i][j]
NLMAX = [1, 3, 5]                 # max_i nl[i][j]  (j = 0,1,2)
IIP = [1, 4, 6]                   # ii padded per j
# F layout: per j block [v(8), iip_j]
FOFF = [0, 8, 40]
F_COLS = 88
# CYT layout: per j block [w(WJ_j), iip_j];  w = (k<NLMAX_j, i, o) k-major
WJ = [9 * NLMAX[j] for j in range(3)]        # 9, 27, 45
IOFF9 = [0, 1, 4]                 # (i,o) offset within a 9-col k-group
CYOFF = [0] + list(accumulate(WJ[j] * IIP[j] for j in range(3)))
CY_COLS = CYOFF[-1]               # 387
# R layout: per j block [(v, k<NLMAX_j), (i,u)=24]
ROFF = [0] + list(accumulate(8 * NLMAX[j] * 24 for j in range(3)))
R_COLS = ROFF[-1]                 # 1728
# rt layout: per j block [(v,k) = t', 72 = (u, 9=(i,o))]
IUO = [0, 8, 32]                  # feature-layout block offsets (output)
RTOFF = [0] + list(accumulate(8 * NLMAX[j] * 72 for j in range(3)))
RT_COLS = RTOFF[-1]               # 5184
G_FOLD = 4                        # t' grouped by 4 -> psum [128, 288] (<=512/bank)
PS_MSG = G_FOLD * 72              # 288


def _cc_layout():
    layout, off = {}, 0
    for _, lo in RS:
        for _, li in RS:
            for lf in range(abs(lo - li), lo + li + 1):
                if (lo, li, lf) not in layout:
                    shp = (2 * lo + 1, 2 * li + 1, 2 * lf + 1)
                    layout[(lo, li, lf)] = (off, shp)
                    off += shp[0] * shp[1] * shp[2]
    return layout, off


CC_LAYOUT, CC_TOTAL = _cc_layout()  # 1225


def _norm_coef():
    nc = np.zeros((3, 3), dtype=np.float64)
    for i, (_, lo) in enumerate(RS):
        ns = sum(mi * (2 * min(lo, li) + 1) for mi, li in RS)
        nc[i, :] = math.sqrt(4 * math.pi) * math.sqrt(2 * lo + 1) / math.sqrt(ns)
    return nc


NORM = _norm_coef()


def build_cc2t(cc: np.ndarray) -> np.ndarray:
    """CC2T [25, CY_COLS]: CYT[e,:] = rsh[e,:] @ CC2T  (NORM folded in).
    Column layout per j: [w=(i, k<NLMAX_j, o), ii<IIP_j]; zero outside
    (k < nl[i][j], ii < nj_j)."""
    cc2t = np.zeros((SH_DIM, CY_COLS), dtype=np.float64)
    for j in range(3):
        for i in range(3):
            nl = NL[i][j]
            for k in range(nl):
                lf = abs(i - j) + k
                off, shp = CC_LAYOUT[(i, j, lf)]
                C = cc[off : off + shp[0] * shp[1] * shp[2]].reshape(shp)
                for o in range(NO[i]):
                    w = k * 9 + IOFF9[i] + o
                    for ii in range(NJ[j]):
                        col = CYOFF[j] + w * IIP[j] + ii
                        cc2t[lf * lf : (lf + 1) * (lf + 1), col] = (
                            NORM[i, j] * C[o, ii, :]
                        )
    return cc2t.astype(np.float32)


def build_w3rep(W3p: np.ndarray) -> np.ndarray:
    """W3rep [100, R_COLS]: R[e,:] = h[e,:] @ W3rep; col (j,v,k,i,u);
    zero for k >= nl[i][j]."""
    w3rep = np.zeros((H, R_COLS), dtype=np.float32)
    for j in range(3):
        for i in range(3):
            nl = NL[i][j]
            p = i * 3 + j
            for v in range(8):
                for k in range(nl):
                    for u in range(8):
                        old = R_OFF_OLD[p] + u * (8 * nl) + v * nl + k
                        col = ROFF[j] + (v * NLMAX[j] + k) * 24 + i * 8 + u
                        w3rep[:, col] = W3p[:, old]
    return w3rep


def rearrange_features(features: np.ndarray) -> np.ndarray:
    """[N, 72] -> [N, 88] ii-padded per-j blocks [v, iip_j]."""
    N = features.shape[0]
    out = np.zeros((N, F_COLS), dtype=np.float32)
    for j in range(3):
        blk = features[:, FEAT_OFF[j] : FEAT_OFF[j + 1]].reshape(N, 8, NJ[j])
        dst = out[:, FOFF[j] : FOFF[j] + 8 * IIP[j]].reshape(N, 8, IIP[j])
        dst[:, :, : NJ[j]] = blk
    return out


def fold_weights(W0, W1, W2, W3):
    s = SWISH_SCALE
    return (
        (W0 / math.sqrt(N_BASIS)).astype(np.float32),
        (s * W1 / math.sqrt(H)).astype(np.float32),
        (s * W2 / math.sqrt(H)).astype(np.float32),
        (s * W3 / math.sqrt(H)).astype(np.float32),
    )


# ----------------- bass program -----------------

def build_program(e_pad: int, n_nodes: int):
    import concourse.bass as bass
    import concourse.tile as tile
    from concourse import bacc, mybir

    f32 = mybir.dt.float32
    bf16 = mybir.dt.bfloat16
    i32 = mybir.dt.int32
    AF = mybir.ActivationFunctionType
    OP = mybir.AluOpType
    AX = mybir.AxisListType

    n_super = e_pad // SUPER
    assert e_pad % SUPER == 0

    nc = bacc.Bacc()

    rshT_d = nc.dram_tensor("rshT", [SH_DIM, e_pad], bf16, kind="ExternalInput")
    rad10_d = nc.dram_tensor("rad10", [N_BASIS, e_pad], f32, kind="ExternalInput")
    fedge_d = nc.dram_tensor("fedge", [e_pad, F_COLS], bf16, kind="ExternalInput")
    amat_d = nc.dram_tensor("amat", [e_pad, SUB], bf16, kind="ExternalInput")
    sid_d = nc.dram_tensor("slotid", [e_pad, 1], i32, kind="ExternalInput")
    w0_d = nc.dram_tensor("W0p", [N_BASIS, H], bf16, kind="ExternalInput")
    w1_d = nc.dram_tensor("W1p", [H, H], bf16, kind="ExternalInput")
    w2_d = nc.dram_tensor("W2p", [H, H], bf16, kind="ExternalInput")
    w3_d = nc.dram_tensor("W3rep", [H, R_COLS], bf16, kind="ExternalInput")
    cc2_d = nc.dram_tensor("CC2T", [SH_DIM, CY_COLS], bf16, kind="ExternalInput")
    csc_d = nc.dram_tensor("cscale", [N_BASIS, 1], f32, kind="ExternalInput")
    cbi_d = nc.dram_tensor("cbias", [N_BASIS, 1], f32, kind="ExternalInput")
    out_d = nc.dram_tensor("out", [n_nodes + 1, FEAT_DIM], f32, kind="ExternalOutput")

    with tile.TileContext(nc) as tc, ExitStack() as ctx:
        cpool = ctx.enter_context(tc.tile_pool(name="consts", bufs=1))
        inpool = ctx.enter_context(tc.tile_pool(name="in", bufs=3))
        hpool = ctx.enter_context(tc.tile_pool(name="h", bufs=2))
        dpool = ctx.enter_context(tc.tile_pool(name="work", bufs=2))
        mpool = ctx.enter_context(tc.tile_pool(name="msg", bufs=3))
        ps_mlp = ctx.enter_context(tc.tile_pool(name="psmlp", bufs=1, space="PSUM"))
        ps_r = ctx.enter_context(tc.tile_pool(name="psr", bufs=1, space="PSUM"))
        ps_cy = ctx.enter_context(tc.tile_pool(name="pscy", bufs=1, space="PSUM"))
        ps_msg = ctx.enter_context(tc.tile_pool(name="psmsg", bufs=1, space="PSUM"))

        w0_s = cpool.tile([N_BASIS, H], bf16)
        w1_s = cpool.tile([H, H], bf16)
        w2_s = cpool.tile([H, H], bf16)
        w3_s = cpool.tile([H, R_COLS], bf16)
        cc2_s = cpool.tile([SH_DIM, CY_COLS], bf16)
        csc_s = cpool.tile([N_BASIS, 1], f32)
        cbi_s = cpool.tile([N_BASIS, 1], f32)
        zero_s = cpool.tile([SUB, FEAT_DIM], f32)
        for t, d in (
            (w0_s, w0_d), (w1_s, w1_d), (w2_s, w2_d), (w3_s, w3_d),
            (cc2_s, cc2_d), (csc_s, csc_d), (cbi_s, cbi_d),
        ):
            nc.sync.dma_start(t[:], d[:])
        nc.vector.memset(zero_s[:], 0.0)

        # zero-init output table (n_nodes + 1 rows)
        n_out = n_nodes + 1
        nfull = n_out // SUB
        if nfull:
            nc.sync.dma_start(
                out_d[: nfull * SUB, :].rearrange("(a p) c -> p a c", p=SUB),
                zero_s[:].unsqueeze(1).broadcast_to((SUB, nfull, FEAT_DIM)),
            )
        rem = n_out - nfull * SUB
        if rem:
            nc.sync.dma_start(out_d[nfull * SUB :, :], zero_s[:rem, :])

        for s in range(n_super):
            e0 = s * SUPER
            # ---- per-super loads ----
            rsh_t = inpool.tile([SH_DIM, SUPER], bf16, tag="rsh")
            nc.sync.dma_start(rsh_t[:], rshT_d[:, e0 : e0 + SUPER])
            rad_t = inpool.tile([N_BASIS, SUPER], f32, tag="rad")
            nc.sync.dma_start(rad_t[:], rad10_d[:, e0 : e0 + SUPER])
            fg_t = inpool.tile([SUB, N_SUB * F_COLS], bf16, tag="fg")
            nc.sync.dma_start(
                fg_t[:],
                fedge_d[e0 : e0 + SUPER, :].rearrange("(c p) f -> p c f", p=SUB),
            )
            am_t = inpool.tile([SUB, N_SUB * SUB], bf16, tag="am")
            nc.sync.dma_start(
                am_t[:],
                amat_d[e0 : e0 + SUPER, :].rearrange("(c p) f -> p c f", p=SUB),
            )
            sid_t = inpool.tile([SUB, N_SUB], i32, tag="sid")
            nc.sync.dma_start(
                sid_t[:],
                sid_d[e0 : e0 + SUPER, 0].rearrange("(c p) -> p c", p=SUB),
            )

            # ---- radial basis + MLP (bf16) ----
            z2_t = hpool.tile([N_BASIS, SUPER], f32, tag="z2")
            nc.scalar.activation(
                z2_t[:], rad_t[:], AF.Square, bias=cbi_s[:], scale=csc_s[:]
            )
            bas_t = hpool.tile([N_BASIS, SUPER], bf16, tag="bas")
            nc.scalar.activation(bas_t[:], z2_t[:], AF.Exp, scale=-1.0)

            h = bas_t
            for li, w_s in enumerate((w0_s, w1_s, w2_s)):
                hp = ps_mlp.tile([H, SUPER], f32, tag="hp", space="PSUM")
                nc.tensor.matmul(hp[:], w_s[:], h[:], start=True, stop=True)
                hn = hpool.tile([H, SUPER], bf16, tag=f"h{li}")
                nc.scalar.activation(hn[:], hp[:], AF.Silu)
                h = hn

            for c in range(N_SUB):
                esl = slice(c * SUB, (c + 1) * SUB)
                P = SUB
                # ---- R = h_c^T @ W3rep -> psum [128, 1728], evac to bf16 ----
                r_ps = ps_r.tile([SUB, R_COLS], f32, tag="r", space="PSUM")
                for n0 in range(0, R_COLS, 512):
                    n1 = min(n0 + 512, R_COLS)
                    nc.tensor.matmul(
                        r_ps[:, n0:n1], h[:, esl], w3_s[:, n0:n1],
                        start=True, stop=True,
                    )
                r_sb = dpool.tile([SUB, R_COLS], bf16, tag="rsb")
                nc.scalar.copy(r_sb[:], r_ps[:])

                # ---- CYT = rsh_c @ CC2T ----
                cy_ps = ps_cy.tile([SUB, CY_COLS], f32, tag="cy", space="PSUM")
                nc.tensor.matmul(
                    cy_ps[:], rsh_t[:, esl], cc2_s[:], start=True, stop=True
                )
                cy_t = dpool.tile([SUB, CY_COLS], bf16, tag="cys")
                nc.scalar.copy(cy_t[:], cy_ps[:])

                fsl = slice(c * F_COLS, (c + 1) * F_COLS)
                fg = fg_t[:, fsl]

                # ---- D-stage ----
                # j=0: D0[v,w9] = F0[v] * CY0[w]
                d0_t = dpool.tile([SUB, 72], bf16, tag="d0")
                nc.vector.tensor_tensor(
                    d0_t[:].rearrange("p (v w) -> p v w", v=8),
                    fg[:, 0:8].rearrange("p (v i) -> p v i", v=8)
                        .broadcast_to((P, 8, 9)),
                    cy_t[:, 0:9].unsqueeze(1).broadcast_to((P, 8, 9)),
                    OP.mult,
                )
                # j=1: pd1[v, w27, ii4] -> tree -> D1[v, w27]
                pd1 = dpool.tile([SUB, 8 * 27 * 4], bf16, tag="pd1")
                nc.vector.tensor_tensor(
                    pd1[:].rearrange("p (v w i) -> p v w i", v=8, w=27),
                    fg[:, 8:40].rearrange("p (v i) -> p v i", v=8)
                        .unsqueeze(2).broadcast_to((P, 8, 27, 4)),
                    cy_t[:, CYOFF[1] : CYOFF[2]]
                        .rearrange("p (w i) -> p w i", w=27)
                        .unsqueeze(1).broadcast_to((P, 8, 27, 4)),
                    OP.mult,
                )
                t1 = dpool.tile([SUB, 8 * 27 * 2], bf16, tag="t1")
                nc.vector.tensor_tensor(
                    t1[:].rearrange("p (g i) -> p g i", i=2),
                    pd1[:].rearrange("p (g i) -> p g i", i=4)[:, :, 0:2],
                    pd1[:].rearrange("p (g i) -> p g i", i=4)[:, :, 2:4],
                    OP.add,
                )
                d1_t = dpool.tile([SUB, 8 * 27], bf16, tag="d1")
                nc.vector.tensor_tensor(
                    d1_t[:],
                    t1[:].rearrange("p (g i) -> p g i", i=2)[:, :, 0],
                    t1[:].rearrange("p (g i) -> p g i", i=2)[:, :, 1],
                    OP.add,
                )
                # j=2: pd2[v, w45, ii6] -> tree -> D2[v, w45]
                pd2 = dpool.tile([SUB, 8 * 45 * 6], bf16, tag="pd2")
                nc.vector.tensor_tensor(
                    pd2[:].rearrange("p (v w i) -> p v w i", v=8, w=45),
                    fg[:, 40:88].rearrange("p (v i) -> p v i", v=8)
                        .unsqueeze(2).broadcast_to((P, 8, 45, 6)),
                    cy_t[:, CYOFF[2] : CYOFF[3]]
                        .rearrange("p (w i) -> p w i", w=45)
                        .unsqueeze(1).broadcast_to((P, 8, 45, 6)),
                    OP.mult,
                )
                ta = dpool.tile([SUB, 8 * 45 * 2], bf16, tag="ta")
                nc.vector.tensor_tensor(
                    ta[:].rearrange("p (g i) -> p g i", i=2),
                    pd2[:].rearrange("p (g i) -> p g i", i=6)[:, :, 0:2],
                    pd2[:].rearrange("p (g i) -> p g i", i=6)[:, :, 2:4],
                    OP.add,
                )
                tb = dpool.tile([SUB, 8 * 45 * 2], bf16, tag="tb")
                nc.vector.tensor_tensor(
                    tb[:].rearrange("p (g i) -> p g i", i=2),
                    ta[:].rearrange("p (g i) -> p g i", i=2),
                    pd2[:].rearrange("p (g i) -> p g i", i=6)[:, :, 4:6],
                    OP.add,
                )
                d2_t = dpool.tile([SUB, 8 * 45], bf16, tag="d2")
                nc.vector.tensor_tensor(
                    d2_t[:],
                    tb[:].rearrange("p (g i) -> p g i", i=2)[:, :, 0],
                    tb[:].rearrange("p (g i) -> p g i", i=2)[:, :, 1],
                    OP.add,
                )
                d_by_j = (d0_t, d1_t, d2_t)

                # ---- R-stage products: rt[(j, t'=(v,k)), (u, 9=(i,o))] ----
                rt = mpool.tile([SUB, RT_COLS], bf16, tag="rt")
                for j in range(3):
                    nlm = NLMAX[j]
                    vk = 8 * nlm
                    dj = d_by_j[j]
                    for i in range(3):
                        no = NO[i]
                        out_ap = (
                            rt[:, RTOFF[j] : RTOFF[j + 1]]
                            .rearrange("p (vk u w) -> p vk u w", u=8, w=9)
                            [:, :, :, IOFF9[i] : IOFF9[i] + no]
                        )
                        r_ap = (
                            r_sb[:, ROFF[j] : ROFF[j + 1]]
                            .rearrange("p (vk iu) -> p vk iu", iu=24)
                            [:, :, i * 8 : i * 8 + 8]
                            .unsqueeze(3)
                            .broadcast_to((P, vk, 8, no))
                        )
                        d_ap = (
                            dj[:]
                            .rearrange("p (vk w) -> p vk w", w=9)
                            [:, :, IOFF9[i] : IOFF9[i] + no]
                            .unsqueeze(2)
                            .broadcast_to((P, vk, 8, no))
                        )
                        nc.vector.tensor_tensor(out_ap, r_ap, d_ap, OP.mult)

                # ---- segment-sum: psum += A^T @ rt (chunks of 576 cols) ----
                msg_ps = ps_msg.tile([SUB, PS_MSG], f32, tag="mps", space="PSUM")
                a_sl = am_t[:, c * SUB : (c + 1) * SUB]
                n_chunks = RT_COLS // PS_MSG  # 9
                for g in range(n_chunks):
                    nc.tensor.matmul(
                        msg_ps[:],
                        a_sl,
                        rt[:, g * PS_MSG : (g + 1) * PS_MSG],
                        start=(g == 0),
                        stop=(g == n_chunks - 1),
                    )
                # fold the 8 sub-chunks; write msg in feature layout (i,u,o)
                msg_t = mpool.tile([SUB, FEAT_DIM], f32, tag="msg")
                for i in range(3):
                    no = NO[i]
                    in_ap = (
                        msg_ps[:]
                        .rearrange("p (t u w) -> p t u w", t=G_FOLD, u=8)
                        [:, :, :, IOFF9[i] : IOFF9[i] + no]
                        .transpose((0, 2, 3, 1))
                    )
                    out_ap = msg_t[:, IUO[i] : IUO[i] + 8 * no].rearrange(
                        "p (u o) -> p u o", u=8
                    )
                    nc.vector.tensor_reduce(out_ap, in_ap, AX.X, OP.add)
                # ---- scatter distinct-dst rows ----
                nc.gpsimd.indirect_dma_start(
                    out=out_d[:],
                    out_offset=bass.IndirectOffsetOnAxis(
                        ap=sid_t[:, c : c + 1], axis=0
                    ),
                    in_=msg_t[:],
                    in_offset=None,
                )

    nc.finalize()
    return nc


# ----------------- host side -----------------

def _prep_consts(cc, W0, W1, W2, W3):
    W0p, W1p, W2p, W3p = fold_weights(W0, W1, W2, W3)
    cc2t = build_cc2t(np.asarray(cc, dtype=np.float64))
    w3rep = build_w3rep(W3p)
    centers = np.linspace(MIN_R, MAX_R, N_BASIS).astype(np.float32)
    spacing = (MAX_R - MIN_R) / (N_BASIS - 1)
    cscale = np.full((N_BASIS, 1), 1.0 / spacing, dtype=np.float32)
    cbias = (-centers / spacing).astype(np.float32).reshape(N_BASIS, 1)
    return (
        W0p.astype(BF16), W1p.astype(BF16), W2p.astype(BF16),
        w3rep.astype(BF16), cc2t.astype(BF16), cscale, cbias,
    )


def pack_edges(dst: np.ndarray, n_nodes: int):
    """Group edges by dst, bin-pack groups into 128-edge tiles (best-fit
    decreasing) so no dst-group straddles a tile. [n_tiles, SUB], -1 pads."""
    import bisect

    order = np.argsort(dst, kind="stable")
    ds = dst[order]
    starts = np.flatnonzero(np.r_[True, ds[1:] != ds[:-1]])
    ends = np.r_[starts[1:], len(ds)]
    runs = sorted(
        ((int(e - s), int(s), int(e)) for s, e in zip(starts, ends)),
        key=lambda r: -r[0],
    )
    assert runs[0][0] <= SUB, f"node with {runs[0][0]} > {SUB} in-edges"
    bins, rems, binidx = [], [], []
    for L, s, e in runs:
        k = bisect.bisect_left(rems, L)
        if k == len(rems):
            bins.append([(s, e)])
            r, bi = SUB - L, len(bins) - 1
        else:
            bi = binidx[k]
            r = rems[k] - L
            del rems[k], binidx[k]
            bins[bi].append((s, e))
        j = bisect.bisect_left(rems, r)
        rems.insert(j, r)
        binidx.insert(j, bi)
    tiles = []
    for b in bins:
        cur = []
        for s, e in b:
            cur.extend(order[s:e].tolist())
        cur.extend([-1] * (SUB - len(cur)))
        tiles.append(cur)
    return np.array(tiles, dtype=np.int64)


def _build_and_maps(edge_index, features, radii, rsh, cc, W0, W1, W2, W3):
    edge_index = np.asarray(edge_index)
    features = np.asarray(features, dtype=np.float32)
    radii = np.asarray(radii, dtype=np.float32)
    rsh = np.asarray(rsh, dtype=np.float32)
    n_nodes = features.shape[0]

    W0p, W1p, W2p, w3rep, cc2t, cscale, cbias = _prep_consts(cc, W0, W1, W2, W3)
    featp = rearrange_features(features)  # [N, 88] f32

    src = edge_index[0].astype(np.int64)
    dst = edge_index[1].astype(np.int64)
    tiles = pack_edges(dst, n_nodes)
    n_tiles = tiles.shape[0]

    tiles_per_core = -(-n_tiles // N_CORES)
    tpc = -(-tiles_per_core // N_SUB) * N_SUB
    e_pad = tpc * SUB

    nc = build_program(e_pad, n_nodes)

    rshT = rsh.T  # [25, E]
    in_maps = []
    for kk in range(N_CORES):
        sel = tiles[kk * tiles_per_core : (kk + 1) * tiles_per_core]
        flat = sel.reshape(-1)
        flat = np.concatenate([flat, np.full(e_pad - flat.size, -1, np.int64)])
        valid = flat >= 0
        idx = np.where(valid, flat, 0)

        rshT_s = np.where(valid[None, :], rshT[:, idx], 0.0).astype(BF16)
        rad_s = np.where(valid, radii[idx], np.float32(1.0))
        rad10_s = np.ascontiguousarray(
            np.broadcast_to(rad_s[None, :], (N_BASIS, e_pad))
        ).astype(np.float32)
        fedge_s = np.where(valid[:, None], featp[src[idx], :], 0.0).astype(BF16)
        dst_v = np.where(valid, dst[idx], -1)

        # per-tile slot assignment
        n_t = e_pad // SUB
        dst_t = dst_v.reshape(n_t, SUB)
        amat = np.zeros((n_t, SUB, SUB), dtype=np.float32)
        sids = np.full((n_t, SUB), n_nodes, dtype=np.int32)
        for t in range(n_t):
            dt = dst_t[t]
            uniq, inv = np.unique(dt, return_inverse=True)
            # drop the -1 pad group if present
            if uniq.size and uniq[0] == -1:
                keep = dt >= 0
                uniq2 = uniq[1:]
                pos = inv - 1
            else:
                keep = np.ones(SUB, bool)
                uniq2 = uniq
                pos = inv
            ns = uniq2.size
            assert ns <= SUB
            sids[t, :ns] = uniq2.astype(np.int32)
            rowsel = np.flatnonzero(keep)
            amat[t, rowsel, pos[rowsel]] = 1.0

        in_maps.append(
            dict(
                rshT=np.ascontiguousarray(rshT_s),
                rad10=rad10_s,
                fedge=np.ascontiguousarray(fedge_s),
                amat=np.ascontiguousarray(
                    amat.reshape(e_pad, SUB).astype(BF16)
                ),
                slotid=np.ascontiguousarray(
                    sids.reshape(e_pad, 1)[: e_pad]
                ).astype(np.int32),
                W0p=W0p, W1p=W1p, W2p=W2p, W3rep=w3rep, CC2T=cc2t,
                cscale=cscale, cbias=cbias,
            )
        )

    return nc, in_maps, n_nodes


def kernel(edge_index, features, radii, rsh, cc, W0, W1, W2, W3):
    from concourse.bass_utils import run_bass_kernel_spmd

    nc, in_maps, n_nodes = _build_and_maps(
        edge_index, features, radii, rsh, cc, W0, W1, W2, W3
    )
    res = run_bass_kernel_spmd(nc, in_maps, core_ids=list(range(N_CORES)))
    out = np.zeros((n_nodes, FEAT_DIM), dtype=np.float32)
    for r in res.results:
        out += r["out"][:n_nodes]
    return out


def _install_ntff_shim():
    """Provide antenv.axon_hooks + the ctypes NTFF hook if absent."""
    import contextlib
    import ctypes
    import sys
    import types

    try:
        from antenv.axon_hooks import get_axon_ntff_profile_hook  # noqa: F401
        return
    except ImportError:
        pass

    holder = {}
    mod = types.ModuleType("antenv.axon_hooks")
    mod.set_axon_ntff_profile_hook = lambda h: holder.__setitem__("h", h)
    mod.get_axon_ntff_profile_hook = lambda: holder.get("h")
    import antenv

    sys.modules["antenv.axon_hooks"] = mod
    antenv.axon_hooks = mod

    so_path = "/opt/axon/libaxon_pjrt.so"
    try:
        lib = ctypes.CDLL(so_path)
    except OSError:
        return
    if not hasattr(lib, "axon_start_nrt_profile"):
        return
    lib.axon_start_nrt_profile.argtypes = [
        ctypes.POINTER(ctypes.c_int64),
        ctypes.c_size_t,
    ]
    lib.axon_start_nrt_profile.restype = ctypes.c_int64
    lib.axon_stop_nrt_profile.argtypes = [ctypes.c_char_p]
    lib.axon_stop_nrt_profile.restype = ctypes.c_int64

    @contextlib.contextmanager
    def _hook(output_dir, device_ids):
        import jax

        jax.devices()
        if device_ids:
            ids = (ctypes.c_int64 * len(device_ids))(*device_ids)
            rc = lib.axon_start_nrt_profile(ids, len(device_ids))
        else:
            rc = lib.axon_start_nrt_profile(None, 0)
        if rc != 0:
            raise RuntimeError(f"axon_start_nrt_profile rc={rc}")
        try:
            yield
        finally:
            n = lib.axon_stop_nrt_profile(str(output_dir).encode())
            print(f"ntff profile: {n} file(s) written to {output_dir}")

    mod.set_axon_ntff_profile_hook(_hook)


def kernel_traced(edge_index, features, radii, rsh, cc, W0, W1, W2, W3,
                  trace_cores=None, tmpdir=None):
    """Run with NTFF tracing; returns BassKernelResults."""
    _install_ntff_shim()
    from concourse import bass_utils

    bass_utils.upload_artifacts = lambda d: f"local:{d}"

    nc, in_maps, n_nodes = _build_and_maps(
        edge_index, features, radii, rsh, cc, W0, W1, W2, W3
    )
    return bass_utils.run_bass_kernel_spmd(
        nc, in_maps, core_ids=list(range(N_CORES)), trace=True,
        trace_cores=trace_cores, tmpdir=tmpdir,
    )
